# revision 55
# baseline (speedup 1.0000x reference)
"""Trainium2 Bass kernel for ChannelMaxPool top-k masking (v5, single-pass).

Reference computation:
  x: (B=32, C=512, H=128, W=128) f32
  scores[b,c] = max |x[b,c,:,:]|
  top-128 channels by score (jax.lax.top_k order: value desc, index asc)
  w[b,k] = exp(s_k) / sum_selected exp(s_j)
  y[b,k,:,:] = x[b, idx_k, :, :] * w[b,k]

Sharding: pure data-parallel, batch split across 8 NeuronCores
(4 samples per core), no communication.

Design (vs the two-pass indirect-DMA baseline at ~589-639us; this
version measures ~463us in a quiet-HBM run, ~500-520 in noisy runs):
  * single pass over HBM: while streaming x for the f32 absmax scan
    (exactness requires f32 scores; bf16 rounding would flip top-k
    order), each tile is also converted to a bf16 stash kept in SBUF
    (16 MiB/sample fits; rotating 38-slot arena via tile-pool tag
    rotation handles WAR deps across samples automatically).
  * the gather re-read from HBM (32 MiB/core of indirect DMA) is
    replaced by one-hot matmuls on the otherwise idle PE: out[k,s] =
    sum_p oh_g[p,k]*stash_g[p,s] accumulated over the 4 channel
    groups in PSUM.  Exactly one nonzero per column -> bit-exact
    gather of the bf16 stash.  HBM traffic drops 176->144 MiB/core.
  * engine split: DVE = absmax reduces + selection math; ACT = stash
    converts + PSUM->SBUF copies (fused with the per-rank softmax
    weight via per-partition scale); PE = score replication + gather
    matmuls; Sync queue = load triggers only; store triggers go on
    the GpSimd queue so they never head-of-line block loads.
  * selection critical path minimized: emitted FIRST at the next
    scan's top so it wins the per-engine priority heaps; the rank
    counts use fused compare+accum_out ops (8 small instructions, no
    comparison buffers); the last scan tile is fine-split 4x so the
    final reduce latency is ~0.6us; one-hot built directly in bf16.
  * emission interleaving: gather of sample b is emitted in t-block
    chunks inside the scan loop of sample b+1 (block T matmuls at
    iter 4T, psum copies at 4T+8) keeping ACT/PE queues in
    dependency order with the arena rotation; SPARE arena slots plus
    the load pool give ~25us of load runway over a ~12us chain.
  * tail (last sample, DMA otherwise idle): spatial blocks 5..7 are
    re-read from HBM by indirect DMA and scaled on the idle DVE,
    in parallel with the PE gathering blocks 0..4 from the stash
    (psum copies alternating DVE/ACT).
"""

import numpy as np

B, C, H, W = 32, 512, 128, 128
S = H * W
K = 128
N_CORES = 8
BL = B // N_CORES

CCH = C // 128           # 4 channel groups of 128
TW = 2048                # scan/store tile width
NST = S // TW            # 8 spatial blocks per group
NTILES = CCH * NST       # 32 scan tiles per sample
SPARE = 6                # extra arena slots of lookahead
GW = 1024                # gather psum chunk width (2 PSUM banks)
FINE = 4                 # sub-splits of the last scan tile per sample
NPC = NTILES + FINE - 1  # partials columns


def _build_nc():
    import concourse.bass as bass
    import concourse.mybir as mybir
    from concourse import bacc
    from concourse.masks import make_identity
    from concourse.tile import TileContext

    f32 = mybir.dt.float32
    bf16 = mybir.dt.bfloat16
    i32 = mybir.dt.int32
    Alu = mybir.AluOpType
    Act = mybir.ActivationFunctionType

    nc = bacc.Bacc()
    x = nc.dram_tensor("x", [BL, C, S], f32, kind="ExternalInput")
    y = nc.dram_tensor("y", [BL, K, S], bf16, kind="ExternalOutput")
    x1 = x[:].rearrange("b c s -> (b c) s")   # rows of 16384 (64 KiB)

    with TileContext(nc) as tc:
        with (
            tc.tile_pool(name="load", bufs=4) as load_pool,
            tc.tile_pool(name="arena", bufs=NTILES + SPARE) as arena,
            tc.tile_pool(name="out", bufs=2) as out_pool,
            tc.tile_pool(name="cmp", bufs=2) as cmp_pool,
            tc.tile_pool(name="big1", bufs=2) as big1,
            tc.tile_pool(name="small", bufs=2) as small,
            tc.tile_pool(name="const", bufs=1) as cpool,
            tc.psum_pool(name="psum", bufs=1) as psum,
        ):
            # ---------------- constants ----------------
            identity = cpool.tile([128, 128], f32, tag="identity")
            make_identity(nc, identity[:])

            ones4 = cpool.tile([CCH, 128], f32, tag="ones4")
            nc.vector.memset(ones4[:], 1.0)
            onescol = cpool.tile([128, 1], f32, tag="onescol")
            nc.vector.memset(onescol[:], 1.0)

            # iota temp borrows a load-pool slot (same tag as scan tiles)
            iotaQ_i = load_pool.tile([128, 128], i32, tag="ld")
            nc.gpsimd.iota(iotaQ_i[:], pattern=[[1, 128]], base=0,
                           channel_multiplier=0)
            iotaQ = cpool.tile([128, 128], f32, tag="iotaQ")
            nc.vector.tensor_copy(iotaQ[:], iotaQ_i[:])

            # M[p, g, q'] = 1.0 if q' < 128g + p else 0.0
            mlt = cpool.tile([128, CCH, C], bf16, tag="mlt")
            nc.gpsimd.memset(mlt[:], 1.0)
            nc.gpsimd.affine_select(
                out=mlt[:], in_=mlt[:], compare_op=Alu.is_gt, fill=0.0,
                base=0, pattern=[[128, CCH], [-1, C]], channel_multiplier=1,
            )
            # blkmask[g', g, q] = 1.0 if g == g'
            blkmask = cpool.tile([CCH, CCH, 128], bf16, tag="blkmask")
            nc.gpsimd.memset(blkmask[:], 1.0)
            nc.gpsimd.affine_select(
                out=blkmask[:], in_=blkmask[:], compare_op=Alu.is_equal,
                fill=0.0, base=0, pattern=[[-1, CCH], [0, 128]],
                channel_multiplier=1,
            )

            # rhs_idx[p, g, :] = (128g, p) -- both bf16-exact; the
            # one-hot matmul against this reconstructs channel ids for
            # the tail's indirect gather
            rhs_idx_i = load_pool.tile([128, CCH, 2], i32, tag="ld")
            nc.gpsimd.iota(rhs_idx_i[:, :, 0], pattern=[[128, CCH]],
                           base=0, channel_multiplier=0)
            nc.gpsimd.iota(rhs_idx_i[:, :, 1], pattern=[[0, CCH]],
                           base=0, channel_multiplier=1)
            rhs_idx = cpool.tile([128, CCH, 2], bf16, tag="rhs_idx")
            nc.vector.tensor_copy(rhs_idx[:], rhs_idx_i[:])

            # ---------------- per-sample helpers ----------------
            def emit_gather_matmuls(st, T):
                """Emit gather matmuls for spatial block T (columns
                [T*TW, (T+1)*TW)) of sample st['b'].  Group-outer
                order (interleaved accumulation groups across PSUM
                banks) so the first matmuls only need ohb[:,0,:] --
                the PE starts ~4.5us before the full rank stage ends.
                The psum->sbuf copies are emitted separately (8 scan
                iters later) so the ACT queue never head-of-line
                blocks on PE."""
                pss = [psum.tile([128, GW], f32, tag="gps", bufs=3,
                                 name=f"gps{T}_{h}")
                       for h in range(TW // GW)]
                for g in range(CCH):
                    for h, ps in enumerate(pss):
                        for c in range(GW // 512):
                            nc.tensor.matmul(
                                out=ps[:, c * 512:(c + 1) * 512],
                                lhsT=st["ohb"][:, g, :],
                                rhs=st["stash"][(T, g)][
                                    :, h * GW + c * 512:h * GW + (c + 1) * 512],
                                start=(g == 0),
                                stop=(g == CCH - 1),
                                skip_group_check=True,
                            )
                st["ps"][T] = pss

            def emit_gather_copy(st, T, split_dve=False):
                """psum -> sbuf bf16 (scaled by per-rank weight), then
                store block T.  Store triggers ride the gpsimd queue
                (the sync queue carries loads).  With split_dve (tail,
                where DVE is idle), the two copies alternate DVE/ACT."""
                b = st["b"]
                for h, ps in enumerate(st["ps"].pop(T)):
                    outb = out_pool.tile([128, GW], bf16, tag="out",
                                         bufs=3)
                    if split_dve and h % 2 == 0:
                        nc.vector.tensor_scalar(
                            out=outb[:], in0=ps[:],
                            scalar1=st["w"][:, 0:1], scalar2=None,
                            op0=Alu.mult,
                        )
                    else:
                        nc.scalar.activation(
                            out=outb[:], in_=ps[:],
                            func=Act.Copy, bias=0.0,
                            scale=st["w"][:, 0:1],
                        )
                    nc.gpsimd.dma_start(
                        out=y[b, :, T * TW + h * GW:T * TW + (h + 1) * GW],
                        in_=outb[:])

            def emit_selection(partials):
                """Part A of selection: rank channels and build the
                one-hot lhsT, per group, so gather matmuls can start
                as early as possible."""
                scores_col = small.tile([128, CCH], f32, tag="scores_col")
                nc.vector.tensor_reduce(
                    out=scores_col[:, :CCH - 1],
                    in_=partials[:, :(CCH - 1) * NST].rearrange(
                        "p (g t) -> p g t", t=NST),
                    axis=mybir.AxisListType.X,
                    op=Alu.max,
                )
                nc.vector.tensor_reduce(
                    out=scores_col[:, CCH - 1:CCH],
                    in_=partials[:, None, (CCH - 1) * NST:NPC],
                    axis=mybir.AxisListType.X,
                    op=Alu.max,
                )

                # replicate scores to all partitions via PE
                sc_t_ps = psum.tile([CCH, 128], f32, tag="sel")
                nc.tensor.transpose(
                    out=sc_t_ps[:], in_=scores_col[:], identity=identity[:])
                sc_t = small.tile([CCH, 128], f32, tag="sc_t_sb", bufs=1)
                nc.vector.tensor_copy(sc_t[:], sc_t_ps[:])
                rhs_blk = small.tile([CCH, CCH, 128], f32, tag="rhs_blk",
                                     bufs=1)
                nc.vector.tensor_tensor(
                    out=rhs_blk[:],
                    in0=sc_t[:, None, :].to_broadcast([CCH, CCH, 128]),
                    in1=blkmask[:],
                    op=Alu.mult,
                )
                b_ps = psum.tile([128, C], f32, tag="selbig")
                nc.tensor.matmul(
                    out=b_ps[:], lhsT=ones4[:], rhs=rhs_blk[:],
                    start=True, stop=True,
                )
                b_sb = big1.tile([128, C], f32, tag="b_sb", bufs=1)
                nc.vector.tensor_copy(b_sb[:], b_ps[:])

                # rank(c) = #{s' > s} + #{c'<c: s'==s}  (exact f32).
                # Each count is one fused DVE op per group: compare
                # (and tie-mask multiply) with the row-sum taken via
                # accum_out -- no comparison buffers, no big reduces.
                rank_col = small.tile([128, CCH], f32, tag="rank_col")
                r2 = small.tile([128, CCH], f32, tag="r2")
                r1 = small.tile([128, CCH], f32, tag="r1")
                trash = cmp_pool.tile([128, C], bf16, tag="trash",
                                      bufs=1)
                # one-hot of rank, built per group directly in bf16
                # (0/1 exact) so gather matmuls for group g can start
                # as soon as group g's rank is known
                ohb = big1.tile([128, CCH, 128], bf16, tag="ohb")
                for g in range(CCH):
                    nc.vector.tensor_scalar(
                        out=trash[:], in0=b_sb[:],
                        scalar1=scores_col[:, g:g + 1], scalar2=0.0,
                        op0=Alu.is_gt, op1=Alu.add,
                        accum_out=r1[:, g:g + 1],
                    )
                    nc.vector.scalar_tensor_tensor(
                        out=trash[:], in0=b_sb[:],
                        scalar=scores_col[:, g:g + 1],
                        in1=mlt[:, g, :],
                        op0=Alu.is_equal, op1=Alu.mult,
                        accum_out=r2[:, g:g + 1],
                    )
                    nc.vector.tensor_tensor(
                        out=rank_col[:, g:g + 1], in0=r1[:, g:g + 1],
                        in1=r2[:, g:g + 1], op=Alu.add)
                    nc.vector.tensor_tensor(
                        out=ohb[:, g, :],
                        in0=iotaQ[:],
                        in1=rank_col[:, g:g + 1].to_broadcast([128, 128]),
                        op=Alu.is_equal,
                    )

                return ohb, {"rank_col": rank_col,
                             "scores_col": scores_col}

            def emit_selection_w(ohb, sel, want_idx=False):
                """Part B of selection: softmax weights (and channel
                ids for the tail).  Emitted after the first gather
                block's matmuls -- only the psum COPIES need w.  The
                idx chain comes first: it only needs ohb, and the
                tail's indirect gathers are gated on it."""
                rank_col = sel["rank_col"]
                scores_col = sel["scores_col"]
                idx_i = None
                if want_idx:
                    idx_ps = psum.tile([128, 2], f32, tag="sel")
                    for g in range(CCH):
                        nc.tensor.matmul(
                            out=idx_ps[:],
                            lhsT=ohb[:, g, :],
                            rhs=rhs_idx[:, g, :],
                            start=(g == 0),
                            stop=(g == CCH - 1),
                        )
                    idx_sb = small.tile([128, 2], f32, tag="idx_sb")
                    nc.vector.tensor_copy(idx_sb[:], idx_ps[:])
                    idx_f = small.tile([128, 1], f32, tag="idx_f")
                    nc.vector.tensor_tensor(
                        out=idx_f[:], in0=idx_sb[:, 0:1],
                        in1=idx_sb[:, 1:2], op=Alu.add)
                    idx_i = small.tile([128, 1], i32, tag="idx_i")
                    nc.vector.tensor_scalar(
                        out=idx_i[:], in0=idx_f[:],
                        scalar1=float((BL - 1) * C), scalar2=None,
                        op0=Alu.add,
                    )
                e_col = small.tile([128, CCH], f32, tag="e_col")
                nc.scalar.activation(
                    out=e_col[:], in_=scores_col[:], func=Act.Exp,
                    bias=0.0, scale=1.0,
                )
                # bf16 es0: the es-of-rank matmul needs a bf16 rhs (its
                # lhsT is the bf16 one-hot); esum (fused accum_out)
                # sums the f32 mask*exp, consistent to ~bf16 rounding
                es0 = small.tile([128, CCH], bf16, tag="es0")
                esum = small.tile([128, 1], f32, tag="esum")
                nc.vector.scalar_tensor_tensor(
                    out=es0[:], in0=rank_col[:], scalar=float(K),
                    in1=e_col[:], op0=Alu.is_lt, op1=Alu.mult,
                    accum_out=esum[:],
                )
                z_ps = psum.tile([128, 4], f32, tag="sel")
                nc.tensor.matmul(
                    out=z_ps[0:1, 0:1], lhsT=onescol[:], rhs=esum[:],
                    start=True, stop=True,
                )
                z_sb = small.tile([1, 1], f32, tag="z_sb")
                nc.vector.tensor_copy(z_sb[:], z_ps[0:1, 0:1])
                zrep_ps = psum.tile([128, 1], f32, tag="sel")
                nc.tensor.matmul(
                    out=zrep_ps[:], lhsT=ones4[0:1, :], rhs=z_sb[:],
                    start=True, stop=True,
                )
                zrep_sb = small.tile([128, 1], f32, tag="zrep_sb")
                nc.vector.tensor_copy(zrep_sb[:], zrep_ps[:])
                zinv = small.tile([128, 1], f32, tag="zinv")
                nc.vector.reciprocal(zinv[:], zrep_sb[:])

                # es of the rank-j channel via PE accumulate
                es_ps = psum.tile([128, 1], f32, tag="sel")
                for g in range(CCH):
                    nc.tensor.matmul(
                        out=es_ps[:],
                        lhsT=ohb[:, g, :],
                        rhs=es0[:, g:g + 1],
                        start=(g == 0),
                        stop=(g == CCH - 1),
                    )
                es_sb = small.tile([128, 1], f32, tag="es_sb")
                nc.vector.tensor_copy(es_sb[:], es_ps[:])
                w_sb = small.tile([128, 1], f32, tag="w_sb")
                nc.vector.tensor_tensor(
                    out=w_sb[:], in0=es_sb[:], in1=zinv[:], op=Alu.mult)
                return w_sb, idx_i

            # ---------------- main pipeline ----------------
            # pend = scanned-but-unselected sample; prev = sample whose
            # gather blocks are being interleaved into the current scan.
            pend = None
            prev = None
            for b in range(BL):
                partials = small.tile([128, NPC], f32, tag="partials")
                stash_tiles = {}
                for j in range(NTILES):
                    t, g = divmod(j, CCH)
                    if j == 0 and pend is not None:
                        # selection of the previous sample emitted first
                        # so its ops win the per-engine ready heaps and
                        # the post-scan critical chain starts ASAP; the
                        # weight chain (part B) is emitted after block
                        # 0's matmuls -- only the psum copies need it
                        ohb, sel = emit_selection(pend["partials"])
                        prev = {"b": pend["b"], "stash": pend["stash"],
                                "ohb": ohb, "ps": {}}
                        emit_gather_matmuls(prev, 0)
                        w_sb, _ = emit_selection_w(ohb, sel)
                        prev["w"] = w_sb
                        pend = None
                    # interleave gather of the previous sample: block T
                    # matmuls at scan iter 4T (arena slots for iter
                    # 4T+4..4T+7 free when block T's matmuls read them),
                    # block T's psum copy at iter 4T+8
                    elif prev is not None and j % CCH == 0:
                        T = j // CCH
                        emit_gather_matmuls(prev, T)
                        if T >= 2:
                            emit_gather_copy(prev, T - 2)
                    ld = load_pool.tile([128, TW], f32, tag="ld")
                    if j == NTILES - 1:
                        # fine-split the last tile: the final absmax
                        # reduce shrinks 2.2us -> 0.6us, so the
                        # selection chain starts that much sooner
                        sw = TW // FINE
                        for u in range(FINE):
                            s0 = t * TW + u * sw
                            nc.sync.dma_start(
                                out=ld[:, u * sw:(u + 1) * sw],
                                in_=x[b, g * 128:(g + 1) * 128,
                                      s0:s0 + sw],
                            )
                            nc.vector.tensor_reduce(
                                out=partials[:, g * NST + t + u:
                                             g * NST + t + u + 1],
                                in_=ld[:, u * sw:(u + 1) * sw],
                                axis=mybir.AxisListType.X,
                                op=Alu.max,
                                apply_absolute_value=True,
                            )
                    else:
                        nc.sync.dma_start(
                            out=ld[:],
                            in_=x[b, g * 128:(g + 1) * 128,
                                  t * TW:(t + 1) * TW],
                        )
                        nc.vector.tensor_reduce(
                            out=partials[:, g * NST + t:g * NST + t + 1],
                            in_=ld[:],
                            axis=mybir.AxisListType.X,
                            op=Alu.max,
                            apply_absolute_value=True,
                        )
                    st_tile = arena.tile([128, TW], bf16, tag="stash")
                    nc.scalar.activation(
                        out=st_tile[:], in_=ld[:], func=Act.Copy,
                        bias=0.0, scale=1.0,
                    )
                    stash_tiles[(t, g)] = st_tile
                if prev is not None:
                    emit_gather_copy(prev, NST - 2)
                    emit_gather_copy(prev, NST - 1)
                    prev = None
                pend = {"b": b, "partials": partials,
                        "stash": stash_tiles}

            # tail: selection + gather of the last sample.  DMA queues
            # are idle here, so spatial blocks NPE..7 are re-read from
            # HBM via indirect DMA (SWDGE) in parallel with the PE
            # gathering blocks 0..NPE-1 from the stash.
            NPE = 6
            bL = pend["b"]
            ohb, sel = emit_selection(pend["partials"])
            prev = {"b": bL, "stash": pend["stash"],
                    "ohb": ohb, "ps": {}}
            emit_gather_matmuls(prev, 0)
            w_sb, idx_i = emit_selection_w(ohb, sel, want_idx=True)
            prev["w"] = w_sb
            # issue ALL indirect gathers back-to-back first so their
            # SWDGE transfers overlap (a store between them would
            # serialize the descriptor generation on the gpsimd queue)
            gts = {}
            for T in range(NPE, NST):
                gt = load_pool.tile([128, TW], f32, tag="ld")
                nc.gpsimd.indirect_dma_start(
                    out=gt[:],
                    out_offset=None,
                    in_=x1,
                    in_offset=bass.IndirectOffsetOnAxis(
                        ap=idx_i[:], axis=0),
                    element_offset=T * TW,
                )
                gts[T] = gt
            # PE path: psum copies stay on ACT (DVE carries the SWDGE
            # scale copies)
            for T in range(1, NPE):
                emit_gather_matmuls(prev, T)
                emit_gather_copy(prev, T - 1)
            emit_gather_copy(prev, NPE - 1)
            for T in range(NPE, NST):
                for h in range(TW // GW):
                    outb = out_pool.tile([128, GW], bf16, tag="out",
                                         bufs=3)
                    nc.vector.tensor_scalar(
                        out=outb[:], in0=gts[T][:, h * GW:(h + 1) * GW],
                        scalar1=w_sb[:, 0:1], scalar2=None,
                        op0=Alu.mult,
                    )
                    nc.gpsimd.dma_start(
                        out=y[bL, :,
                              T * TW + h * GW:T * TW + (h + 1) * GW],
                        in_=outb[:])

    if not nc.is_finalized():
        nc.finalize()
    return nc


_NC_CACHE = None


def _get_nc():
    global _NC_CACHE
    if _NC_CACHE is None:
        _NC_CACHE = _build_nc()
    return _NC_CACHE


def _run(x, trace=False):
    from concourse.bass_utils import run_bass_kernel_spmd

    nc = _get_nc()
    xr = np.ascontiguousarray(x, dtype=np.float32).reshape(N_CORES, BL, C, S)
    in_maps = [{"x": xr[c]} for c in range(N_CORES)]
    res = run_bass_kernel_spmd(nc, in_maps, list(range(N_CORES)), trace=trace)
    out = np.empty((B, K, H, W), dtype=np.float32)
    for c in range(N_CORES):
        out[c * BL:(c + 1) * BL] = np.asarray(
            res.results[c]["y"]).astype(np.float32).reshape(BL, K, H, W)
    return out, res


def kernel(x):
    out, _ = _run(x, trace=False)
    return out
